# revision 7
# baseline (speedup 1.0000x reference)
"""Pairwise cosine similarity [8192,1024]x[8192,1024] -> [8192,8192] on 8 trn2 cores.

Sharding: 4x2 grid. Core (i,j) takes input1 rows [2048*i, 2048*(i+1)) and
input2 rows [4096*j, 4096*(j+1)), computes its [2048, 4096] output block.
All cores run one SPMD program; the host slices inputs and assembles blocks.

Device program (per core), v4 -- all-fp16 PE pipeline:
  - y tiles: ACT square w/ accum_out -> sqrt -> recip/scale on DVE, cast
    fp16 -> PE transpose (fp16 = 1 cyc/row) -> PSUM -> copy into resident
    y^T [128, 8, 4096].
  - x tiles: fast path is load -> DVE cast fp16 -> PE transpose (no norm
    dependency, so matmuls start early). Norms are computed later from the
    fp16 copy and folded into the PSUM->SBUF output copy (per-partition).
  - fp16 matmuls (1 cyc/row at N=512; LDWEIGHTS hides under the stream)
    accumulate 8 K-slabs into PSUM; DVE scales by 1/||x|| while copying
    PSUM->SBUF as fp16; DMA out fp16, host upcasts.
  Queues: loads+stores on SP; ACT carries only square/sqrt (nothing coupled
  to matmul completion); DVE carries scales/casts/copies.
"""

import numpy as np

import concourse.bacc as bacc
import concourse.bass as bass
import concourse.masks as masks
import concourse.mybir as mybir
import concourse.tile as tile
from concourse.bass_utils import run_bass_kernel_spmd

P = 128
D = 1024
KD = D // P  # 8 k-slabs of the contraction dim
N_FULL = 8192
M_FULL = 8192
GRID_N, GRID_M = 4, 2
N_LOC = N_FULL // GRID_N  # 2048
M_LOC = M_FULL // GRID_M  # 4096
EPS = 1e-8
F32 = mybir.dt.float32
F16 = mybir.dt.float16

# Set by test harness to capture profiling info; harness-default is off.
TRACE = False
LAST_RESULT = None


def build(n_loc=N_LOC, m_loc=M_LOC, n_cores=8):
    """Build + compile the SPMD program for one core's [n_loc, m_loc] block."""
    nt_tiles = n_loc // P          # 16: x tiles / output row-tiles
    my_tiles = m_loc // P          # 32: y tiles
    mc_chunks = m_loc // 512       # 8: output column chunks (1 PSUM bank each)
    ypc = 512 // P                 # 4: y tiles per chunk

    nc = bacc.Bacc("TRN2", target_bir_lowering=False, debug=False,
                   num_devices=n_cores)
    x_d = nc.dram_tensor("x", [n_loc, D], F32, kind="ExternalInput").ap()
    y_d = nc.dram_tensor("y", [m_loc, D], F32, kind="ExternalInput").ap()
    o_d = nc.dram_tensor("o", [n_loc, m_loc], F16, kind="ExternalOutput").ap()

    with tile.TileContext(nc) as tc:
        with (
            tc.tile_pool(name="persist", bufs=1) as persist,
            tc.tile_pool(name="stage", bufs=5) as stage,
            tc.tile_pool(name="sqy", bufs=2) as sqyp,
            tc.tile_pool(name="sqx", bufs=2) as sqxp,
            tc.tile_pool(name="yf16", bufs=6) as yf16,
            tc.tile_pool(name="xf16", bufs=nt_tiles) as xf16,
            tc.tile_pool(name="small", bufs=8) as small,
            tc.tile_pool(name="outp", bufs=6) as outp,
            tc.tile_pool(name="pst", bufs=3, space=bass.MemorySpace.PSUM) as pst,
            tc.tile_pool(name="pso", bufs=4, space=bass.MemorySpace.PSUM) as pso,
        ):
            identf = persist.tile([P, P], F32)
            masks.make_identity(nc, identf[:])
            ident = persist.tile([P, P], F16)
            nc.vector.tensor_copy(ident[:], identf[:])

            # Transposed fp16 operands, resident for the whole kernel.
            # Separate tiles => granular deps, first matmuls start early.
            xts = [persist.tile([P, KD, P], F16, name=f"xt{i}", tag=f"xt{i}")
                   for i in range(nt_tiles)]
            yts = [persist.tile([P, KD, 512], F16, name=f"yc{c}", tag=f"yc{c}")
                   for c in range(mc_chunks)]
            # Per-row-tile 1/||x|| (applied at the PSUM->SBUF copy).
            rinvx = [persist.tile([P, 1], F32, name=f"rx{i}", tag=f"rx{i}")
                     for i in range(nt_tiles)]
            # fp16 x tiles persist until the deferred norm pass reads them.
            xss = {}

            tile_seq = [0]

            def load(src_rows):
                ts = stage.tile([P, D], F32, name="ts", tag="ts")
                nc.sync.dma_start(ts[:], src_rows)
                return ts

            def transpose_into(dst_ap, src16):
                # 8 PE transposes [128,128] -> one PSUM bank -> one copy out.
                # Alternate the copy between DVE and ACT so back-to-back
                # tiles pipeline.
                tile_seq[0] += 1
                ps = pst.tile([P, KD, P], F16, name="ps", tag="ps")
                for k in range(KD):
                    nc.tensor.transpose(ps[:, k, :],
                                        src16[:, k * P:(k + 1) * P],
                                        ident[:])
                if tile_seq[0] % 2 == 0:
                    nc.vector.tensor_copy(dst_ap, ps[:])
                else:
                    nc.scalar.copy(dst_ap, ps[:])

            def y_prep(t):
                # Normalize y tile t, cast fp16, PE-transpose into its chunk.
                ts = load(y_d[t * P:(t + 1) * P, :])
                sq = sqyp.tile([P, D], F32, name="sqt", tag="sqt")
                ss = small.tile([P, 1], F32, name="ss", tag="ss")
                nc.scalar.activation(sq[:], ts[:],
                                     mybir.ActivationFunctionType.Square,
                                     accum_out=ss[:])
                rv = small.tile([P, 1], F32, name="rv", tag="rv")
                nc.scalar.sqrt(rv[:], ss[:])
                nc.vector.tensor_scalar_max(rv[:], rv[:], EPS)
                nc.vector.reciprocal(rv[:], rv[:])
                ys = yf16.tile([P, D], F16, name="ys", tag="ys")
                nc.vector.tensor_scalar_mul(ys[:], ts[:], rv[:])
                transpose_into(
                    yts[t // ypc][:, :, (t % ypc) * P:(t % ypc + 1) * P],
                    ys)

            def x_fast(t):
                # Load + cast only: the transpose has no norm dependency.
                ts = load(x_d[t * P:(t + 1) * P, :])
                xs = xf16.tile([P, D], F16, name="xs", tag="xs")
                nc.vector.tensor_copy(xs[:], ts[:])
                xss[t] = xs
                transpose_into(xts[t][:], xs)

            def x_norm(t):
                # Deferred: 1/||x|| from the fp16 copy (error ~1e-4, fine).
                xs = xss.pop(t)
                sq = sqxp.tile([P, D], F16, name="sqx", tag="sqx")
                ss = small.tile([P, 1], F32, name="ssx", tag="ssx")
                nc.scalar.activation(sq[:], xs[:],
                                     mybir.ActivationFunctionType.Square,
                                     accum_out=ss[:])
                nc.scalar.sqrt(rinvx[t][:], ss[:])
                nc.vector.tensor_scalar_max(rinvx[t][:], rinvx[t][:], EPS)
                nc.vector.reciprocal(rinvx[t][:], rinvx[t][:])

            def mm_group(mc, nt):
                po = pso.tile([P, 512], F32, name="po", tag="po")
                for k in range(KD):
                    nc.tensor.matmul(
                        po[:],
                        xts[nt][:, k, :],
                        yts[mc][:, k, :],
                        start=(k == 0),
                        stop=(k == KD - 1))
                ot = outp.tile([P, 512], F16, name="ot", tag="ot")
                # Fold 1/||x|| in while copying PSUM->SBUF (cast to fp16).
                nc.vector.tensor_scalar_mul(ot[:], po[:], rinvx[nt][:])
                nc.sync.dma_start(
                    o_d[nt * P:(nt + 1) * P, mc * 512:(mc + 1) * 512],
                    ot[:])

            # --- emission schedule: keep every engine queue flowing ---
            y_next = [0]

            def prep_y_upto(t_end):
                while y_next[0] < min(t_end, my_tiles):
                    y_prep(y_next[0])
                    y_next[0] += 1

            prep_y_upto(ypc)                   # chunk 0's y tiles
            x_fast(0)
            if nt_tiles > 1:
                x_fast(1)
            x_next = min(2, nt_tiles)
            xn_next = 0
            for mc in range(mc_chunks):
                prep_y_upto((mc + 1) * ypc)    # ensure this chunk is queued
                for nt in range(nt_tiles):
                    # The deferred norm must be emitted before the group
                    # whose output copy reads it (deps follow program order).
                    while xn_next <= nt and xn_next < nt_tiles and mc == 0:
                        x_norm(xn_next)
                        xn_next += 1
                    mm_group(mc, nt)
                    # Trickle remaining preps between groups. x transposes
                    # must finish within chunk 0; the next chunk's y tiles
                    # are needed sooner than the last x tiles, so interleave
                    # a y prep after every other x prep.
                    if x_next < nt_tiles:
                        x_fast(x_next)
                        x_next += 1
                        if x_next % 2 == 0 and y_next[0] < my_tiles:
                            y_prep(y_next[0])
                            y_next[0] += 1
                    elif y_next[0] < my_tiles:
                        y_prep(y_next[0])
                        y_next[0] += 1

    nc.compile()
    return nc


_NC = None


def _get_nc():
    global _NC
    if _NC is None:
        _NC = build()
    return _NC


def kernel(input1, input2):
    global LAST_RESULT
    input1 = np.ascontiguousarray(np.asarray(input1, dtype=np.float32))
    input2 = np.ascontiguousarray(np.asarray(input2, dtype=np.float32))
    nc = _get_nc()
    in_maps = []
    for i in range(GRID_N):
        for j in range(GRID_M):
            in_maps.append({
                "x": input1[i * N_LOC:(i + 1) * N_LOC],
                "y": input2[j * M_LOC:(j + 1) * M_LOC],
            })
    res = run_bass_kernel_spmd(nc, in_maps, list(range(GRID_N * GRID_M)),
                               trace=TRACE)
    LAST_RESULT = res
    out = np.empty((N_FULL, M_FULL), dtype=np.float32)
    idx = 0
    for i in range(GRID_N):
        for j in range(GRID_M):
            out[i * N_LOC:(i + 1) * N_LOC,
                j * M_LOC:(j + 1) * M_LOC] = res.results[idx]["o"]
            idx += 1
    return out
